# revision 27
# baseline (speedup 1.0000x reference)
"""Trainium2 Bass kernel for SAGAN-style self-attention (nn_Attention).

Reference computation (per batch b):
  f = Wf @ x + bf            [32, N]   (N = 64*64 = 4096 pixels)
  g = Wg @ y + bg            [32, N]
  h = Wh @ y + bh            [64, N]
  s[m, n] = sum_c g[c, m] f[c, n]
  beta = softmax(s, axis=n)
  o[m, c] = sum_n beta[m, n] h[c, n]
  out = gamma * o^T + x      [64, N]

Sharding: 8 cores = 4 batches x 2 query-halves. Each core computes the full
softmax rows for its 2048 queries (m) against all 4096 keys (n). The key
axis is permuted host-side so the core's own query half is always columns
0:2048 -> identical SPMD program on all cores.

On-chip algorithm (St orientation: keys on partitions, m on free dim):
  St[n, m] = f[:, n].T @ g          (K=32, 4-way row-tiled concurrent MMs)
  E = exp(St)                        (ACT, PSUM->SBUF, bf16 out)
  O'[c|Z, m] = [gamma*hT | 1].T @ E  (K=128, accumulated over 32 n-chunks,
                                      chunk-major so LDWEIGHTS amortizes
                                      over the 4 m-banks)
  out[c, m] = O'[c, m] / Z[m] + x[c, m]
The kernel is ACT(exp)-bound: 8.4M exps/core ~= 66us of Scalar engine.
Everything else (DMAs, projections, both matmul passes, normalize) is
scheduled to hide behind the exp stream.
gamma is folded into Wh host-side; softmax max-subtraction is skipped
(|s| <= ~8, exp is safe in fp32).
"""
import numpy as np
import ml_dtypes

import bass_rust
import concourse.bass as bass

import concourse.mybir as mybir
import concourse.tile as tile
from concourse.bass_utils import run_bass_kernel_spmd


F32 = mybir.dt.float32
F32R = mybir.dt.float32r
BF16 = mybir.dt.bfloat16
AF = mybir.ActivationFunctionType

B, C, N = 4, 64, 4096
M = N // 2              # queries per core
CH = 64
MCH = 512               # m per matmul (one PSUM bank)


def split_multi_waits(nc, max_waits=1):
    """This walrus build supports a single sync-wait per instruction; spill
    extras onto fresh same-engine NOPs placed right before the instruction."""
    n_spill = 0
    for f in nc.m.functions:
        for bb in f.blocks:
            out = []
            changed = False
            for inst in bb.instructions:
                si = inst.sync_info
                if si is not None and len(si.on_wait) > max_waits:
                    waits = list(si.on_wait)
                    spill, keep = waits[:-max_waits], waits[-max_waits:]
                    for j in range(0, len(spill), max_waits):
                        n_spill += 1
                        out.append(
                            mybir.InstNoOp(
                                name=f"I-waitspill-{n_spill}",
                                engine=inst.engine,
                                bass_nofuse=True,
                                sync_info=mybir.SyncInfo(
                                    on_wait=spill[j : j + max_waits], on_update=[]
                                ),
                            )
                        )
                    inst.sync_info = bass_rust.SyncInfo(
                        on_update=list(si.on_update), on_wait=keep
                    )
                    changed = True
                out.append(inst)
            if changed:
                bb.instructions = out
    return n_spill


def build_kernel():
    nc = bass.Bass("TRN2", target_bir_lowering=False, debug=False, num_devices=8)

    # bf16 inputs are pre-augmented with a ones row (bias fold) and
    # pre-permuted so this core's queries are always columns 0:M.
    xab = nc.dram_tensor("xab", [C + 1, N], BF16, kind="ExternalInput").ap()
    yab = nc.dram_tensor("yab", [C + 1, N], BF16, kind="ExternalInput").ap()
    xres = nc.dram_tensor("xres", [C, M], F32, kind="ExternalInput").ap()
    wf4 = nc.dram_tensor("wf4", [C + 1, 128], BF16, kind="ExternalInput").ap()
    wg4 = nc.dram_tensor("wg4", [C + 1, 128], BF16, kind="ExternalInput").ap()
    wh = nc.dram_tensor("wh", [C + 1, CH], BF16, kind="ExternalInput").ap()
    out = nc.dram_tensor("out", [C, M], F32, kind="ExternalOutput").ap()

    with tile.TileContext(nc) as tc:
        with (
            tc.tile_pool(name="persist", bufs=1) as sb,
            tc.tile_pool(name="epool", bufs=40) as ep,
            tc.tile_pool(name="scratch", bufs=2) as sc,
            tc.tile_pool(name="pst", bufs=2, space="PSUM") as pst,
            tc.tile_pool(name="pacc", bufs=1, space="PSUM") as pacc,
        ):
            # --- tiny dummy ln+exp: trigger the ACT table load ASAP.  Ln
            # first selects the natural_log_exp_and_others set, which also
            # covers every later Exp AND the tail's 1/Z = exp(-ln Z), so the
            # ~2.7us table load happens exactly once ---
            dm = sc.tile([1, 1], F32, tag="dummy")
            nc.vector.memset(dm[:], 1.0)
            dml = sc.tile([1, 1], F32, tag="dummy")
            nc.scalar.activation(dml[:], dm[:], AF.Ln)
            dme = sc.tile([1, 1], F32, tag="dummy")
            nc.scalar.activation(dme[:], dml[:], AF.Exp)

            # --- PE warmup off a memset tile (no DMA bandwidth stolen from
            # the real inputs; clock gate opens while they're in flight) ---
            wwarm_sb = sb.tile([128, 512], BF16)
            nc.vector.memset(wwarm_sb[:], 1.0)
            wps = pst.tile([128, 512], F32, tag="st")
            for i in range(3):
                nc.tensor.matmul(
                    wps[:], wwarm_sb[:, 0:128], wwarm_sb[:],
                    start=True, stop=True,
                )

            # --- input DMAs: two queues, critical-first; xres (chain-time
            # residual, 512KB) is deferred into quad 0 so the early HBM
            # bandwidth all goes to x/y ---
            wg4_sb = sb.tile([C + 1, 128], BF16)
            wf4_sb = sb.tile([C + 1, 128], BF16)
            wh_sb = sb.tile([C + 1, CH], BF16)
            y_sb = sb.tile([C + 1, N], BF16)
            x_sb = sb.tile([C + 1, N], BF16)
            nc.sync.dma_start(wg4_sb[:], wg4[:])
            nc.sync.dma_start(y_sb[:, 0:512], yab[:, 0:512])
            nc.gpsimd.dma_start(wf4_sb[:], wf4[:])
            nc.gpsimd.dma_start(x_sb[:, 0:512], xab[:, 0:512])
            nc.gpsimd.dma_start(wh_sb[:], wh[:])
            # bulk pieces ride the Scalar queue: its descriptors only run
            # after the ACT table load, so the critical first pieces above
            # get the HBM bandwidth to themselves
            nc.scalar.dma_start(y_sb[:, 512:2048], yab[:, 512:2048])
            nc.scalar.dma_start(x_sb[:, 512:2048], xab[:, 512:2048])
            nc.scalar.dma_start(x_sb[:, 2048:4096], xab[:, 2048:4096])
            nc.scalar.dma_start(y_sb[:, 2048:4096], yab[:, 2048:4096])

            # --- persistent SBUF for projections ---
            # g4: [128, M] = 4 stacked copies of g over the core's queries.
            # f4: [128, N] = 4 stacked copies of f over all keys.
            # hT_all: 32 chunks of [128, 65]; cols 65k..65k+64 = gamma*hT of
            # key chunk k (keys on partitions), col 65k+64 = ones (Z col).
            g4_sb = sb.tile([128, M], BF16)
            f4_sb = sb.tile([128, N], BF16)
            hT_all = sb.tile([128, 32 * (CH + 1)], BF16)
            # Z/ones column of hT_all (bf16 memset on the strided slice)
            onesdst = hT_all[:].rearrange("p (k e) -> p k e", k=32)[:, :, 64:65]
            nc.vector.memset(onesdst, 1.0)

            # 4-bank PSUM scratch ring for all projections; the O' output
            # accumulator reuses partitions 0..64 of the same banks later
            # (slice-level deps order projections before the first O' MM).
            acc = pacc.tile([128, 2048], F32, tag="acc")
            ring = {"i": 0}

            def proj_g4(mj):
                bk = ring["i"] % 4
                ring["i"] += 1
                dst = acc[:, bass.ts(bk, MCH)]
                nc.tensor.matmul(
                    dst, wg4_sb[:], y_sb[:, bass.ts(mj, MCH)],
                    start=True, stop=True,
                )
                nc.vector.tensor_copy(g4_sb[:, bass.ts(mj, MCH)], dst)

            def proj_f4(q):
                bk = ring["i"] % 4
                ring["i"] += 1
                dst = acc[:, bass.ts(bk, MCH)]
                nc.tensor.matmul(
                    dst, wf4_sb[:], x_sb[:, bass.ts(q, MCH)],
                    start=True, stop=True,
                )
                nc.vector.tensor_copy(f4_sb[:, bass.ts(q, MCH)], dst)

            def proj_hT(t):
                # chunks 8t..8t+7 -> hT_all (keys on partitions)
                bk = ring["i"] % 4
                ring["i"] += 1
                dst = acc[:, bass.ts(bk, MCH)]
                for u in range(8):
                    k = 8 * t + u
                    nc.tensor.matmul(
                        dst[:, bass.ds(64 * u, 64)],
                        y_sb[:, bass.ts(k, 128)], wh_sb[:],
                        start=True, stop=True,
                    )
                h_dst = hT_all[:].rearrange("p (k e) -> p k e", k=32)[
                    :, 8 * t : 8 * t + 8, 0:64
                ]
                nc.vector.tensor_copy(
                    h_dst, dst.rearrange("p (a b) -> p a b", a=8)
                )

            # Deferred projection pieces, emitted inside quad 0 (the pacc
            # banks must all be free before O' accumulation begins).
            deferred = [
                lambda: proj_g4(1),
                lambda: proj_f4(1),
                lambda: proj_g4(2),
                lambda: proj_g4(3),
                lambda: proj_f4(2),
                lambda: proj_hT(0),
                lambda: proj_f4(3),
                lambda: proj_hT(1),
                lambda: proj_f4(4),
                lambda: proj_hT(2),
                lambda: proj_f4(5),
                lambda: proj_hT(3),
                lambda: proj_f4(6),
                lambda: proj_f4(7),
            ]

            # first pieces (gate the start of the main loop)
            proj_g4(0)
            proj_f4(0)

            # --- chain: per-m-bank normalize + residual + store ---
            ones64b = sb.tile([1, CH], BF16)
            nc.vector.memset(ones64b[:], 1.0)

            def _t(nm, shape, dt, n=4):
                return [
                    sc.tile(shape, dt, tag=f"{nm}{i}", name=f"{nm}{i}", bufs=1)
                    for i in range(n)
                ]

            lnz = _t("lnz", [1, MCH], F32)
            r0b8 = _t("r0b8", [1, MCH], BF16)
            rb_sb = _t("rbsb", [CH, MCH], F32)
            o_sb = _t("osb", [CH, MCH], F32)

            def emit_chain(b):
                # 1/Z = exp(-ln Z) straight off the PSUM Z row on ACT (idle
                # once the exps are done), broadcast via K=1 bf16 matmul,
                # normalize on DVE.  The +xres residual rides the output DMA:
                # x was pre-written to `out` in DRAM during quad 1 and the
                # gpsimd software-DGE accumulates gamma*o/Z on top.
                cs = bass.ts(b, MCH)
                nc.scalar.activation(lnz[b][:], acc[CH : CH + 1, cs], AF.Ln)
                nc.scalar.activation(
                    r0b8[b][:], lnz[b][:], AF.Exp, scale=-1.0
                )
                rb_ps = pst.tile([CH, MCH], F32, tag="st", name=f"rbps{b}")
                nc.tensor.matmul(
                    rb_ps[:], ones64b[:], r0b8[b][:], start=True, stop=True,
                )
                nc.vector.tensor_copy(rb_sb[b][:], rb_ps[:])
                nc.vector.tensor_mul(o_sb[b][:], acc[0:CH, cs], rb_sb[b][:])
                nc.gpsimd.dma_start(
                    out[:, cs], o_sb[b][:], accum_op=mybir.AluOpType.add
                )

            # --- main loop ---
            # 32 flat steps s = (q, mj).  Per step: 4 row-tiled St MMs, two
            # exps, then the O' batch for the PREVIOUS step's bank (its
            # e-tiles finished an exp-period ago, so the in-order PE queue
            # never blocks waiting on a fresh exp), plus at most one quad-0
            # catchup MM.  Quad 0 has no O' (the pacc banks still hold
            # projection scratch); its 16 chunk-MMs are spread one per step
            # over steps 5..20.  start/stop flags are tracked per bank.
            started = [False, False, False, False]

            def oprime_mm(c, b, pair):
                # pair = the two e-tiles [chunks 4q+2h+j] holding bank b's m
                hp, j = (c % 4) // 2, c % 2
                nc.tensor.matmul(
                    acc[0 : CH + 1, bass.ts(b, MCH)],
                    hT_all[:, bass.ds(65 * c, 65)],
                    pair[hp][:, bass.ts(j, MCH)],
                    start=not started[b], stop=(c == 31),
                )
                started[b] = True

            def oprime_batch(bq, bmj, btiles):
                for c in range(4 * bq, 4 * bq + 4):
                    oprime_mm(c, bmj, btiles)

            cat_sched = {
                5 + 4 * c + b: (c, b) for c in range(4) for b in range(4)
            }

            e_q0 = []
            prev = None  # (q, mj, this step's two e-tiles)
            for s in range(32):
                q, mj = divmod(s, 4)
                sts = []
                for hh in range(2):
                    st = pst.tile([128, 1024], F32, tag="st")
                    sts.append(st)
                    for rr in range(2):
                        r = 2 * hh + rr
                        nc.tensor.matmul(
                            st[:, bass.ts(rr, MCH)],
                            f4_sb[
                                bass.ds(32 * r, 32), bass.ts(4 * q + r, 128)
                            ],
                            g4_sb[bass.ds(32 * r, 32), bass.ts(mj, MCH)],
                            start=True, stop=True,
                            tile_position=(32 * r, 0),
                        )
                ecur = []
                for hh in range(2):
                    e_t = ep.tile([128, 1024], BF16, tag="e")
                    nc.scalar.activation(e_t[:], sts[hh][:], AF.Exp)
                    ecur.append(e_t)
                if q == 0:
                    e_q0 += ecur
                    # drain deferred projections behind quad-0 exps
                    for _ in range(4):
                        if deferred:
                            deferred.pop(0)()
                if s == 4:
                    # pre-write the residual x into `out` (DRAM->DRAM); the
                    # tail chains accumulate on top of it
                    for b in range(4):
                        nc.sync.dma_start(
                            out[:, bass.ts(b, MCH)], xres[:, bass.ts(b, MCH)]
                        )
                if prev is not None:
                    oprime_batch(*prev)
                if s in cat_sched:
                    c, b = cat_sched[s]
                    oprime_mm(c, b, e_q0[2 * b : 2 * b + 2])
                if 4 <= s <= 30:
                    # HAM anti-throttle filler: dummy weight loads keep the
                    # PE array active through the ~25% idle slack of each
                    # ACT-bound step, so the clock gate stays at 8/8
                    for _ in range(5):
                        nc.tensor.ldweights(wwarm_sb[:, 0:128])
                prev = (q, mj, ecur) if q >= 1 else None
            # tail: last bank's O', then all four chains (every PSUM read of
            # acc stays behind the last PE write to it)
            oprime_batch(*prev)
            for b in range(4):
                emit_chain(b)

    split_multi_waits(nc)
    return nc


def make_in_maps(x, y, Wf, bf, Wg, bg, Wh, bh, gamma):
    x = np.asarray(x, dtype=np.float32).reshape(B, C, N)
    y = np.asarray(y, dtype=np.float32).reshape(B, C, N)
    bf16 = ml_dtypes.bfloat16
    gamma = np.asarray(gamma, dtype=np.float32).reshape(-1)[0]
    wf4 = np.tile(
        np.concatenate([np.asarray(Wf).T, np.asarray(bf)[None, :]], 0), (1, 4)
    ).astype(bf16)
    wg4 = np.tile(
        np.concatenate([np.asarray(Wg).T, np.asarray(bg)[None, :]], 0), (1, 4)
    ).astype(bf16)
    # gamma folded into the h projection (the Z/ones column stays 1.0)
    wh = (
        gamma
        * np.concatenate([np.asarray(Wh).T, np.asarray(bh)[None, :]], 0)
    ).astype(bf16)
    onesr = np.ones((1, N), np.float32)

    in_maps = []
    for core in range(8):
        b, half = core // 2, core % 2
        mine = slice(half * M, half * M + M)
        other = slice((1 - half) * M, (1 - half) * M + M)
        xa = np.concatenate([x[b][:, mine], x[b][:, other]], axis=1)
        ya = np.concatenate([y[b][:, mine], y[b][:, other]], axis=1)
        xab = np.concatenate([xa, onesr], axis=0).astype(bf16)
        yab = np.concatenate([ya, onesr], axis=0).astype(bf16)
        in_maps.append(
            {
                "xab": np.ascontiguousarray(xab),
                "yab": np.ascontiguousarray(yab),
                "xres": np.ascontiguousarray(x[b][:, mine]),
                "wf4": wf4, "wg4": wg4, "wh": wh,
            }
        )
    return in_maps


def assemble_output(results):
    o = np.empty((B, C, N), np.float32)
    for core in range(8):
        b, half = core // 2, core % 2
        o[b][:, half * M : half * M + M] = results[core]["out"]
    return o.reshape(B, C, 64, 64)


_NC_CACHE = {}


def run(trace=False, **inputs):
    if "nc" not in _NC_CACHE:
        _NC_CACHE["nc"] = build_kernel()
    nc = _NC_CACHE["nc"]
    in_maps = make_in_maps(**inputs)
    res = run_bass_kernel_spmd(nc, in_maps, list(range(8)), trace=trace)
    return assemble_output(res.results), res


def kernel(**inputs):
    out, _ = run(trace=False, **inputs)
    return out


# revision 30
# speedup vs baseline: 1.2406x; 1.2406x over previous
"""Trainium2 Bass kernel for SAGAN-style self-attention (nn_Attention).

Reference computation (per batch b):
  f = Wf @ x + bf            [32, N]   (N = 64*64 = 4096 pixels)
  g = Wg @ y + bg            [32, N]
  h = Wh @ y + bh            [64, N]
  s[m, n] = sum_c g[c, m] f[c, n]
  beta = softmax(s, axis=n)
  o[m, c] = sum_n beta[m, n] h[c, n]
  out = gamma * o^T + x      [64, N]

Sharding: 8 cores = 4 batches x 2 query-halves. Each core computes the full
softmax rows for its 2048 queries (m) against all 4096 keys (n). The key
axis is permuted host-side so the core's own query half is always columns
0:2048 -> identical SPMD program on all cores.

On-chip algorithm (St orientation: keys on partitions, m on free dim):
  St[n, m] = f[:, n].T @ g          (K=32, 4-way row-tiled concurrent MMs)
  E = exp(St)                        (ACT, PSUM->SBUF, bf16 out)
  O'[c|Z, m] = [gamma*hT | 1].T @ E  (K=128, accumulated over 32 n-chunks,
                                      chunk-major so LDWEIGHTS amortizes
                                      over the 4 m-banks)
  out[c, m] = O'[c, m] / Z[m] + x[c, m]
The kernel is ACT(exp)-bound: 8.4M exps/core ~= 66us of Scalar engine.
Everything else (DMAs, projections, both matmul passes, normalize) is
scheduled to hide behind the exp stream.
gamma is folded into Wh host-side; softmax max-subtraction is skipped
(|s| <= ~8, exp is safe in fp32).
"""
import numpy as np
import ml_dtypes

import bass_rust
import concourse.bass as bass

import concourse.mybir as mybir
import concourse.tile as tile
from concourse.bass_utils import run_bass_kernel_spmd


F32 = mybir.dt.float32
F32R = mybir.dt.float32r
BF16 = mybir.dt.bfloat16
AF = mybir.ActivationFunctionType

B, C, N = 4, 64, 4096
M = N // 2              # queries per core
CH = 64
MCH = 512               # m per matmul (one PSUM bank)


def split_multi_waits(nc, max_waits=1):
    """This walrus build supports a single sync-wait per instruction; spill
    extras onto fresh same-engine NOPs placed right before the instruction."""
    n_spill = 0
    for f in nc.m.functions:
        for bb in f.blocks:
            out = []
            changed = False
            for inst in bb.instructions:
                si = inst.sync_info
                if si is not None and len(si.on_wait) > max_waits:
                    waits = list(si.on_wait)
                    spill, keep = waits[:-max_waits], waits[-max_waits:]
                    for j in range(0, len(spill), max_waits):
                        n_spill += 1
                        out.append(
                            mybir.InstNoOp(
                                name=f"I-waitspill-{n_spill}",
                                engine=inst.engine,
                                bass_nofuse=True,
                                sync_info=mybir.SyncInfo(
                                    on_wait=spill[j : j + max_waits], on_update=[]
                                ),
                            )
                        )
                    inst.sync_info = bass_rust.SyncInfo(
                        on_update=list(si.on_update), on_wait=keep
                    )
                    changed = True
                out.append(inst)
            if changed:
                bb.instructions = out
    return n_spill


def build_kernel():
    nc = bass.Bass("TRN2", target_bir_lowering=False, debug=False, num_devices=8)

    # bf16 inputs are pre-augmented with a ones row (bias fold) and
    # pre-permuted so this core's queries are always columns 0:M.
    xab = nc.dram_tensor("xab", [C + 1, N], BF16, kind="ExternalInput").ap()
    yab = nc.dram_tensor("yab", [C + 1, N], BF16, kind="ExternalInput").ap()
    xres = nc.dram_tensor("xres", [C, M], F32, kind="ExternalInput").ap()
    wf4 = nc.dram_tensor("wf4", [C + 1, 128], BF16, kind="ExternalInput").ap()
    wg4 = nc.dram_tensor("wg4", [C + 1, 128], BF16, kind="ExternalInput").ap()
    wh = nc.dram_tensor("wh", [C + 1, CH], BF16, kind="ExternalInput").ap()
    out = nc.dram_tensor("out", [C, M], F32, kind="ExternalOutput").ap()

    with tile.TileContext(nc) as tc:
        with (
            tc.tile_pool(name="persist", bufs=1) as sb,
            tc.tile_pool(name="epool", bufs=40) as ep,
            tc.tile_pool(name="scratch", bufs=2) as sc,
            tc.tile_pool(name="pst", bufs=2, space="PSUM") as pst,
            tc.tile_pool(name="pacc", bufs=1, space="PSUM") as pacc,
        ):
            # --- tiny dummy exp: trigger the ACT table load ASAP ---
            dm = sc.tile([1, 1], F32, tag="dummy")
            nc.vector.memset(dm[:], 1.0)
            dme = sc.tile([1, 1], F32, tag="dummy")
            nc.scalar.activation(dme[:], dm[:], AF.Exp)

            # --- PE warmup off a memset tile (no DMA bandwidth stolen from
            # the real inputs; clock gate opens while they're in flight) ---
            wwarm_sb = sb.tile([128, 512], BF16)
            nc.vector.memset(wwarm_sb[:], 1.0)
            wps = pst.tile([128, 512], F32, tag="st")
            for i in range(3):
                nc.tensor.matmul(
                    wps[:], wwarm_sb[:, 0:128], wwarm_sb[:],
                    start=True, stop=True,
                )

            # --- input DMAs: two queues, critical-first; xres (chain-time
            # residual, 512KB) is deferred into quad 0 so the early HBM
            # bandwidth all goes to x/y ---
            wg4_sb = sb.tile([C + 1, 128], BF16)
            wf4_sb = sb.tile([C + 1, 128], BF16)
            wh_sb = sb.tile([C + 1, CH], BF16)
            y_sb = sb.tile([C + 1, N], BF16)
            x_sb = sb.tile([C + 1, N], BF16)
            nc.sync.dma_start(wg4_sb[:], wg4[:])
            nc.sync.dma_start(y_sb[:, 0:512], yab[:, 0:512])
            nc.gpsimd.dma_start(wf4_sb[:], wf4[:])
            nc.gpsimd.dma_start(x_sb[:, 0:512], xab[:, 0:512])
            nc.gpsimd.dma_start(wh_sb[:], wh[:])
            nc.sync.dma_start(y_sb[:, 512:2048], yab[:, 512:2048])
            nc.sync.dma_start(x_sb[:, 512:2048], xab[:, 512:2048])
            nc.sync.dma_start(x_sb[:, 2048:4096], xab[:, 2048:4096])
            nc.gpsimd.dma_start(y_sb[:, 2048:4096], yab[:, 2048:4096])

            # --- persistent SBUF for projections ---
            # g4: [128, M] = 4 stacked copies of g over the core's queries.
            # f4: [128, N] = 4 stacked copies of f over all keys.
            # hT_all: 32 chunks of [128, 65]; cols 65k..65k+64 = gamma*hT of
            # key chunk k (keys on partitions), col 65k+64 = ones (Z col).
            g4_sb = sb.tile([128, M], BF16)
            f4_sb = sb.tile([128, N], BF16)
            hT_all = sb.tile([128, 32 * (CH + 1)], BF16)
            # Z/ones column of hT_all (bf16 memset on the strided slice)
            onesdst = hT_all[:].rearrange("p (k e) -> p k e", k=32)[:, :, 64:65]
            nc.vector.memset(onesdst, 1.0)

            # 4-bank PSUM scratch ring for all projections; the O' output
            # accumulator reuses partitions 0..64 of the same banks later
            # (slice-level deps order projections before the first O' MM).
            acc = pacc.tile([128, 2048], F32, tag="acc")
            ring = {"i": 0}

            def proj_g4(mj):
                bk = ring["i"] % 4
                ring["i"] += 1
                dst = acc[:, bass.ts(bk, MCH)]
                nc.tensor.matmul(
                    dst, wg4_sb[:], y_sb[:, bass.ts(mj, MCH)],
                    start=True, stop=True,
                )
                nc.vector.tensor_copy(g4_sb[:, bass.ts(mj, MCH)], dst)

            def proj_f4(q):
                bk = ring["i"] % 4
                ring["i"] += 1
                dst = acc[:, bass.ts(bk, MCH)]
                nc.tensor.matmul(
                    dst, wf4_sb[:], x_sb[:, bass.ts(q, MCH)],
                    start=True, stop=True,
                )
                nc.vector.tensor_copy(f4_sb[:, bass.ts(q, MCH)], dst)

            def proj_hT(t):
                # chunks 8t..8t+7 -> hT_all (keys on partitions)
                bk = ring["i"] % 4
                ring["i"] += 1
                dst = acc[:, bass.ts(bk, MCH)]
                for u in range(8):
                    k = 8 * t + u
                    nc.tensor.matmul(
                        dst[:, bass.ds(64 * u, 64)],
                        y_sb[:, bass.ts(k, 128)], wh_sb[:],
                        start=True, stop=True,
                    )
                h_dst = hT_all[:].rearrange("p (k e) -> p k e", k=32)[
                    :, 8 * t : 8 * t + 8, 0:64
                ]
                nc.vector.tensor_copy(
                    h_dst, dst.rearrange("p (a b) -> p a b", a=8)
                )

            # Deferred projection pieces, emitted inside quad 0 (the pacc
            # banks must all be free before O' accumulation begins).
            deferred = [
                lambda: proj_g4(1),
                lambda: proj_f4(1),
                lambda: proj_g4(2),
                lambda: proj_g4(3),
                lambda: proj_f4(2),
                lambda: proj_hT(0),
                lambda: proj_f4(3),
                lambda: proj_hT(1),
                lambda: proj_f4(4),
                lambda: proj_hT(2),
                lambda: proj_f4(5),
                lambda: proj_hT(3),
                lambda: proj_f4(6),
                lambda: proj_f4(7),
            ]

            # first pieces (gate the start of the main loop)
            proj_g4(0)
            proj_f4(0)

            # --- chain: per-m-bank normalize + residual + store ---
            ones64b = sb.tile([1, CH], BF16)
            nc.vector.memset(ones64b[:], 1.0)

            def _t(nm, shape, dt, n=4):
                return [
                    sc.tile(shape, dt, tag=f"{nm}{i}", name=f"{nm}{i}", bufs=1)
                    for i in range(n)
                ]

            lnz = _t("lnz", [1, MCH], F32)
            r0b8 = _t("r0b8", [1, MCH], BF16)
            rb_sb = _t("rbsb", [CH, MCH], F32)
            o_sb = _t("osb", [CH, MCH], F32)

            def emit_chain(b):
                # 1/Z = exp(-ln Z) straight off the PSUM Z row on ACT (idle
                # once the exps are done), broadcast via K=1 bf16 matmul,
                # normalize on DVE.  The +xres residual rides the output DMA:
                # x was pre-written to `out` in DRAM during quad 1 and the
                # gpsimd software-DGE accumulates gamma*o/Z on top.
                cs = bass.ts(b, MCH)
                nc.scalar.activation(lnz[b][:], acc[CH : CH + 1, cs], AF.Ln)
                nc.scalar.activation(
                    r0b8[b][:], lnz[b][:], AF.Exp, scale=-1.0
                )
                rb_ps = pst.tile([CH, MCH], F32, tag="st", name=f"rbps{b}")
                nc.tensor.matmul(
                    rb_ps[:], ones64b[:], r0b8[b][:], start=True, stop=True,
                )
                nc.vector.tensor_copy(rb_sb[b][:], rb_ps[:])
                nc.vector.tensor_mul(o_sb[b][:], acc[0:CH, cs], rb_sb[b][:])
                nc.gpsimd.dma_start(
                    out[:, cs], o_sb[b][:], accum_op=mybir.AluOpType.add
                )

            # --- main loop ---
            # 32 flat steps s = (q, mj).  Per step: 4 row-tiled St MMs, two
            # exps, then the O' batch for the PREVIOUS step's bank (its
            # e-tiles finished an exp-period ago, so the in-order PE queue
            # never blocks waiting on a fresh exp), plus at most one quad-0
            # catchup MM.  Quad 0 has no O' (the pacc banks still hold
            # projection scratch); its 16 chunk-MMs are spread one per step
            # over steps 5..20.  start/stop flags are tracked per bank.
            started = [False, False, False, False]

            def oprime_mm(c, b, pair):
                # pair = the two e-tiles [chunks 4q+2h+j] holding bank b's m
                hp, j = (c % 4) // 2, c % 2
                nc.tensor.matmul(
                    acc[0 : CH + 1, bass.ts(b, MCH)],
                    hT_all[:, bass.ds(65 * c, 65)],
                    pair[hp][:, bass.ts(j, MCH)],
                    start=not started[b], stop=(c == 31),
                )
                started[b] = True

            def oprime_batch(bq, bmj, btiles):
                for c in range(4 * bq, 4 * bq + 4):
                    oprime_mm(c, bmj, btiles)

            cat_sched = {
                5 + 4 * c + b: (c, b) for c in range(4) for b in range(4)
            }

            e_q0 = []
            prev = None  # (q, mj, this step's two e-tiles)
            for s in range(32):
                q, mj = divmod(s, 4)
                sts = []
                for hh in range(2):
                    st = pst.tile([128, 1024], F32, tag="st")
                    sts.append(st)
                    for rr in range(2):
                        r = 2 * hh + rr
                        nc.tensor.matmul(
                            st[:, bass.ts(rr, MCH)],
                            f4_sb[
                                bass.ds(32 * r, 32), bass.ts(4 * q + r, 128)
                            ],
                            g4_sb[bass.ds(32 * r, 32), bass.ts(mj, MCH)],
                            start=True, stop=True,
                            tile_position=(32 * r, 0),
                        )
                ecur = []
                for hh in range(2):
                    e_t = ep.tile([128, 1024], BF16, tag="e")
                    nc.scalar.activation(e_t[:], sts[hh][:], AF.Exp)
                    ecur.append(e_t)
                if q == 0:
                    e_q0 += ecur
                    # drain deferred projections behind quad-0 exps
                    for _ in range(4):
                        if deferred:
                            deferred.pop(0)()
                if s == 4:
                    # pre-write the residual x into `out` (DRAM->DRAM); the
                    # tail chains accumulate on top of it
                    for b in range(4):
                        nc.sync.dma_start(
                            out[:, bass.ts(b, MCH)], xres[:, bass.ts(b, MCH)]
                        )
                if prev is not None:
                    oprime_batch(*prev)
                if s in cat_sched:
                    c, b = cat_sched[s]
                    oprime_mm(c, b, e_q0[2 * b : 2 * b + 2])
                prev = (q, mj, ecur) if q >= 1 else None
            # tail: last bank's O', then all four chains (every PSUM read of
            # acc stays behind the last PE write to it)
            oprime_batch(*prev)
            for b in range(4):
                emit_chain(b)

    split_multi_waits(nc)
    return nc


def make_in_maps(x, y, Wf, bf, Wg, bg, Wh, bh, gamma):
    x = np.asarray(x, dtype=np.float32).reshape(B, C, N)
    y = np.asarray(y, dtype=np.float32).reshape(B, C, N)
    bf16 = ml_dtypes.bfloat16
    gamma = np.asarray(gamma, dtype=np.float32).reshape(-1)[0]
    wf4 = np.tile(
        np.concatenate([np.asarray(Wf).T, np.asarray(bf)[None, :]], 0), (1, 4)
    ).astype(bf16)
    wg4 = np.tile(
        np.concatenate([np.asarray(Wg).T, np.asarray(bg)[None, :]], 0), (1, 4)
    ).astype(bf16)
    # gamma folded into the h projection (the Z/ones column stays 1.0)
    wh = (
        gamma
        * np.concatenate([np.asarray(Wh).T, np.asarray(bh)[None, :]], 0)
    ).astype(bf16)
    onesr = np.ones((1, N), np.float32)

    in_maps = []
    for core in range(8):
        b, half = core // 2, core % 2
        mine = slice(half * M, half * M + M)
        other = slice((1 - half) * M, (1 - half) * M + M)
        xa = np.concatenate([x[b][:, mine], x[b][:, other]], axis=1)
        ya = np.concatenate([y[b][:, mine], y[b][:, other]], axis=1)
        xab = np.concatenate([xa, onesr], axis=0).astype(bf16)
        yab = np.concatenate([ya, onesr], axis=0).astype(bf16)
        in_maps.append(
            {
                "xab": np.ascontiguousarray(xab),
                "yab": np.ascontiguousarray(yab),
                "xres": np.ascontiguousarray(x[b][:, mine]),
                "wf4": wf4, "wg4": wg4, "wh": wh,
            }
        )
    return in_maps


def assemble_output(results):
    o = np.empty((B, C, N), np.float32)
    for core in range(8):
        b, half = core // 2, core % 2
        o[b][:, half * M : half * M + M] = results[core]["out"]
    return o.reshape(B, C, 64, 64)


_NC_CACHE = {}


def run(trace=False, **inputs):
    if "nc" not in _NC_CACHE:
        _NC_CACHE["nc"] = build_kernel()
    nc = _NC_CACHE["nc"]
    in_maps = make_in_maps(**inputs)
    res = run_bass_kernel_spmd(nc, in_maps, list(range(8)), trace=trace)
    return assemble_output(res.results), res


def kernel(**inputs):
    out, _ = run(trace=False, **inputs)
    return out
